# revision 36
# baseline (speedup 1.0000x reference)
"""ClassAttention (CaiT-style) Trainium2 Bass kernel.

Problem: B=32, N=4096, C=768, H=12, D=64.
  q = (x[:,0] @ Wq + bq) * D**-0.5          (CLS token only)
  k = x @ Wk + bk ; v = x @ Wv + bv
  attn = softmax(q k^T) ; x_cls = (attn v) @ Wp + bp
Returns (x_cls [B,1,C], attn [B,H,1,N]).

Algebraic restructuring (single query per (b,h) means k/v never need to be
materialized):
  scores[b,h,n] = x[b,n] . R[b][:,h]  with  R[b] = Wk @ Qmat[b]   (768 x 12)
      (the bk contribution is constant per (b,h) row -> softmax-invariant)
  x_cls[b,h]    = (sum_n p[b,h,n] x[b,n]) @ Wv[:,hD:(h+1)D] + bv_h
This reduces ~309 GFLOP of dense QKV matmuls to ~5 GFLOP; the kernel is
memory-bound on streaming x once (402 MB).

Sharding: pure data-parallel over batch, 4 batches per core x 8 cores.

Per-core dataflow (2-stage software pipeline over 512-token blocks):
  stage A: DMA x block [128,4,768] -> PE transposes (fp32r identity matmuls,
           full PE rate) -> PSUM -> DVE/ACT copies -> xT [128,6,512]
  stage B: scores s^T = R^T @ xT (fp32r, F=512) -> [12,512] PSUM -> ACT exp
           (no max-sub needed; |s| < ~2 by construction) -> pu [12,4096] +
           per-block partial sums; PE transposes pu tiles -> puT [128,12] ->
           weighted-sum matmuls (puT stationary, x natural moving, fp32r)
           accumulate w [12,768] in PSUM across the batch.
  batch tail: DMA out unnormalized attn + denominators + raw w.
Host epilogue (in kernel()): attn normalization and the tiny per-head
Wv / Wp projections (~0.3% of total FLOPs) in numpy.

fp32r note: the PE's raw-fp32 single-pass mode (4x faster than fp32's
2x-half-speed-pass scheme for the moving operand). All producers feeding
fp32r matmuls are declared float32r so engines round on write (BIR verifier
requirement). Measured end-to-end rel err ~1.5e-4.
"""

import numpy as np

B, N, C = 32, 4096, 768
H, D = 12, 64
SCALE = D ** -0.5
NCORES = 8
NB = B // NCORES          # batches per core = 4
CCH = C // 128            # channel chunks = 6
NT = N // 128             # token tiles per batch = 32
BLK = 512                 # scores block (tokens)
NBLK = N // BLK           # 8
TPB = BLK // 128          # token tiles per block = 4


def _build_nc():
    import concourse.bass as bass
    import concourse.tile as tile
    from concourse import bacc, mybir
    from concourse.masks import make_identity
    from contextlib import ExitStack

    f32 = mybir.dt.float32
    f32r = mybir.dt.float32r
    AF = mybir.ActivationFunctionType

    nc = bacc.Bacc("TRN2", debug=False, target_bir_lowering=False)

    x_d = nc.dram_tensor("x", [NB, N, C], f32r, kind="ExternalInput").ap()
    wq_d = nc.dram_tensor("Wq", [C, C], f32, kind="ExternalInput").ap()
    wk_d = nc.dram_tensor("Wk", [C, C], f32, kind="ExternalInput").ap()
    bq_d = nc.dram_tensor("bq", [C], f32, kind="ExternalInput").ap()
    attn_d = nc.dram_tensor("out_attn", [NB, H, N], f32, kind="ExternalOutput").ap()
    sums_d = nc.dram_tensor("out_sums", [NB, H], f32, kind="ExternalOutput").ap()
    w_d = nc.dram_tensor("out_w", [NB, H, C], f32, kind="ExternalOutput").ap()

    with tile.TileContext(nc) as tc, ExitStack() as ctx:
        const = ctx.enter_context(tc.tile_pool(name="const", bufs=1))

        ident = const.tile([128, 128], f32)
        make_identity(nc, ident)
        identr = const.tile([128, 128], f32r)
        nc.vector.tensor_copy(out=identr, in_=ident)

        bq_sb = const.tile([128, CCH], f32)

        r_sb = const.tile([128, CCH, NB * H], f32r)    # R for all 4 batches (fp32r for fast PE path)

        # ---------------- init: q, Qmat, Wk^T, R ----------------
        with (
            tc.tile_pool(name="init", bufs=1) as initp,
            tc.tile_pool(name="init_ps", bufs=1, space="PSUM") as initps,
            tc.tile_pool(name="init_tps", bufs=2, space="PSUM") as inittps,
        ):
            x0n_sb = initp.tile([NB, C], f32r)
            nc.sync.dma_start(out=x0n_sb, in_=x_d[:, 0, :])
            nc.sync.dma_start(out=bq_sb, in_=bq_d.rearrange("(c p) -> p c", p=128))
            wk_sb = initp.tile([128, CCH, C], f32)
            nc.sync.dma_start(out=wk_sb, in_=wk_d.rearrange("(c p) j -> p c j", p=128))
            wq_sb = initp.tile([128, CCH, C], f32)
            nc.sync.dma_start(out=wq_sb, in_=wq_d.rearrange("(c p) j -> p c j", p=128))
            x0_sb = initp.tile([128, CCH, NB], f32)
            for cc in range(CCH):
                t0 = inittps.tile([128, NB], f32r, tag="tps", name=f"t0_{cc}")
                nc.tensor.transpose(t0, x0n_sb[:, cc * 128:(cc + 1) * 128], identr[:NB, :NB])
                nc.vector.tensor_copy(out=x0_sb[:, cc, :], in_=t0)

            # Wk^T: wkT[p, jc, i] = Wk[i, jc*128+p]
            wkT_sb = initp.tile([128, CCH, C], f32)
            for ic in range(CCH):
                for jc in range(CCH):
                    tp = inittps.tile([128, 128], f32, tag="tps")
                    nc.tensor.transpose(tp, wk_sb[:, ic, jc * 128:(jc + 1) * 128], ident)
                    nc.vector.tensor_copy(out=wkT_sb[:, jc, ic * 128:(ic + 1) * 128], in_=tp)

            # q^T[p, jc, b] = (x0 @ Wq + bq) * SCALE
            q_ps = initps.tile([128, CCH, NB], f32, tag="qps")
            for jc in range(CCH):
                for ic in range(CCH):
                    nc.tensor.matmul(
                        q_ps[:, jc, :],
                        wq_sb[:, ic, jc * 128:(jc + 1) * 128],
                        x0_sb[:, ic, :],
                        start=(ic == 0),
                        stop=(ic == CCH - 1),
                    )
            q_sb = initp.tile([128, CCH, NB], f32)
            for jc in range(CCH):
                nc.vector.tensor_scalar(
                    out=q_sb[:, jc, :],
                    in0=q_ps[:, jc, :],
                    scalar1=bq_sb[:, jc:jc + 1],
                    scalar2=SCALE,
                    op0=mybir.AluOpType.add,
                    op1=mybir.AluOpType.mult,
                )

            # Qmat[p, jc, b*H+h] = q_sb[p, jc, b] if (jc*128+p) in head-h block else 0
            qm_sb = initp.tile([128, CCH, NB * H], f32)
            nc.vector.memset(qm_sb, 0.0)
            for b in range(NB):
                for h in range(H):
                    p0 = (h % 2) * 64
                    nc.vector.tensor_copy(
                        out=qm_sb[p0:p0 + 64, h // 2, b * H + h:b * H + h + 1],
                        in_=q_sb[p0:p0 + 64, h // 2, b:b + 1],
                    )

            # R[p, ic, b*H+h] = sum_j Wk[ic*128+p, j] Qmat[j, b*H+h]
            r_ps = initps.tile([128, CCH, NB * H], f32, tag="rps")
            for ic in range(CCH):
                for jc in range(CCH):
                    nc.tensor.matmul(
                        r_ps[:, ic, :],
                        wkT_sb[:, jc, ic * 128:(ic + 1) * 128],
                        qm_sb[:, jc, :],
                        start=(jc == 0),
                        stop=(jc == CCH - 1),
                    )
            nc.vector.tensor_copy(out=r_sb, in_=r_ps)

        # ---------------- steady: stream x, scores, exp, weighted sums ----------
        xpool = ctx.enter_context(tc.tile_pool(name="x", bufs=5))
        xtpool = ctx.enter_context(tc.tile_pool(name="xT", bufs=3))
        pupool = ctx.enter_context(tc.tile_pool(name="pu", bufs=2))
        smallp = ctx.enter_context(tc.tile_pool(name="small", bufs=4))
        putpool = ctx.enter_context(tc.tile_pool(name="puts", bufs=4))
        ps_xt = ctx.enter_context(tc.tile_pool(name="ps_xt", bufs=4, space="PSUM"))
        ps_s = ctx.enter_context(tc.tile_pool(name="ps_s", bufs=2, space="PSUM"))
        ps_w = ctx.enter_context(tc.tile_pool(name="ps_w", bufs=1, space="PSUM"))

        batch_state = {}

        def stage_a(b, blk):
            """DMA one 512-token block, PE-transpose it, copy xT to SBUF."""
            xq = xpool.tile([128, TPB, C], f32r, tag="xq", name=f"xq_{b}_{blk}")
            nc.sync.dma_start(
                out=xq,
                in_=x_d[b, blk * BLK:(blk + 1) * BLK, :].rearrange(
                    "(t p) c -> p t c", p=128
                ),
            )
            xt_sb = xtpool.tile(
                [128, CCH, BLK], f32r, tag="xt", name=f"xt_{b}_{blk}"
            )
            for tt in range(TPB):
                for hf in range(2):
                    xt_ps = ps_xt.tile(
                        [128, 3, 128], f32r, tag="xtps", name=f"xtps_{b}_{blk}_{tt}_{hf}"
                    )
                    for k in range(3):
                        cc = hf * 3 + k
                        nc.tensor.transpose(
                            xt_ps[:, k, :], xq[:, tt, cc * 128:(cc + 1) * 128], identr
                        )
                    dst = xt_sb[:, hf * 3:hf * 3 + 3, tt * 128:(tt + 1) * 128]
                    if (blk * TPB + tt) % 2 == 1:
                        nc.scalar.copy(out=dst, in_=xt_ps[:])
                    else:
                        nc.vector.tensor_copy(out=dst, in_=xt_ps)
            return xq, xt_sb

        def stage_b(b, blk, xq, xt_sb):
            """Scores, exp, puT transposes, weighted-sum accumulation."""
            st = batch_state[b]
            pu_sb, psums, w_ps = st["pu"], st["psums"], st["w"]
            s_ps = ps_s.tile([12, BLK], f32, tag="sps", name=f"sps_{b}_{blk}")
            for cc in range(CCH):
                nc.tensor.matmul(
                    s_ps,
                    r_sb[:, cc, b * H:(b + 1) * H],
                    xt_sb[:, cc, :],
                    start=(cc == 0),
                    stop=(cc == CCH - 1),
                )
            nc.scalar.activation(
                out=pu_sb[:, blk * BLK:(blk + 1) * BLK],
                in_=s_ps,
                func=AF.Exp,
                accum_out=psums[:, blk:blk + 1],
            )
            for tt in range(TPB):
                t = blk * TPB + tt
                put_ps = ps_s.tile(
                    [128, H], f32, tag="sps", name=f"putps_{b}_{blk}_{tt}"
                )
                nc.tensor.transpose(
                    put_ps, pu_sb[:, t * 128:(t + 1) * 128], ident[:12, :12]
                )
                put_sb = putpool.tile(
                    [128, H], f32r, tag="puts", name=f"puts_{b}_{blk}_{tt}"
                )
                nc.vector.tensor_copy(out=put_sb, in_=put_ps)
                for hf in range(2):
                    nc.tensor.matmul(
                        w_ps[hf],
                        put_sb,
                        xq[:, tt, hf * 384:(hf + 1) * 384],
                        start=(t == 0),
                        stop=(t == NT - 1),
                    )

        def batch_tail(b):
            """Unnormalized attn + raw w out (host normalizes and projects),
            plus softmax denominators."""
            st = batch_state.pop(b)
            pu_sb, psums, w_ps = st["pu"], st["psums"], st["w"]
            nc.sync.dma_start(out=attn_d[b], in_=pu_sb)
            s_sum = smallp.tile([12, 1], f32, tag="ssum", name=f"ssum_{b}")
            nc.vector.reduce_sum(out=s_sum, in_=psums, axis=mybir.AxisListType.X)
            nc.sync.dma_start(
                out=sums_d[b:b + 1, :].rearrange("o h -> h o"), in_=s_sum
            )
            w_sb = smallp.tile([12, C], f32, tag="wsb", name=f"wsb_{b}")
            for hf in range(2):
                nc.vector.tensor_copy(
                    out=w_sb[:, hf * 384:(hf + 1) * 384], in_=w_ps[hf]
                )
            nc.sync.dma_start(out=w_d[b], in_=w_sb)

        # 2-stage software pipeline over all (batch, block) pairs: transposes
        # of step i+1 are emitted before scores/wsum of step i, so the PE can
        # run compute for step i while DVE/ACT copies catch up on step i+1.
        steps = [(b, blk) for b in range(NB) for blk in range(NBLK)]
        pend = None
        for b, blk in steps:
            if blk == 0:
                batch_state[b] = {
                    "pu": pupool.tile([12, N], f32, tag="pu", name=f"pu_{b}"),
                    "psums": smallp.tile(
                        [12, NBLK], f32, tag="psums", name=f"psums_{b}"
                    ),
                    "w": [
                        ps_w.tile([12, 384], f32, tag=f"w{i}", name=f"w_ps{i}_{b}")
                        for i in range(2)
                    ],
                }
            cur = (b, blk, *stage_a(b, blk))
            if pend is not None:
                pb, pblk, pxq, pxt = pend
                stage_b(pb, pblk, pxq, pxt)
                if pblk == NBLK - 1:
                    batch_tail(pb)
            pend = cur
        pb, pblk, pxq, pxt = pend
        stage_b(pb, pblk, pxq, pxt)
        batch_tail(pb)

    nc.compile()
    return nc


_NC = None


def _get_nc():
    global _NC
    if _NC is None:
        _NC = _build_nc()
    return _NC


def _run(inputs, trace=False, trace_cores=None):
    from concourse.bass_utils import run_bass_kernel_spmd

    def f32c(a):
        return np.ascontiguousarray(np.asarray(a, dtype=np.float32))

    x = f32c(inputs["x"])
    dev_weights = {k: f32c(inputs[k]) for k in ("Wq", "Wk", "bq")}

    nc = _get_nc()
    in_maps = []
    for i in range(NCORES):
        m = {"x": np.ascontiguousarray(x[i * NB:(i + 1) * NB])}
        m.update(dev_weights)
        in_maps.append(m)
    kwargs = {}
    if trace:
        kwargs.update(trace=True, trace_cores=trace_cores or [0])
    res = run_bass_kernel_spmd(nc, in_maps, core_ids=list(range(NCORES)), **kwargs)
    attn = np.concatenate([r["out_attn"] for r in res.results], axis=0)
    sums = np.concatenate([r["out_sums"] for r in res.results], axis=0)
    w_raw = np.concatenate([r["out_w"] for r in res.results], axis=0)
    attn = attn / sums[:, :, None]
    # host epilogue: per-head Wv projection + final Wp projection (tiny FLOPs)
    w_norm = w_raw / sums[:, :, None]                      # [B, H, C]
    wv = f32c(inputs["Wv"]).reshape(C, H, D)
    x_cat = np.einsum("bhc,chd->bhd", w_norm, wv).reshape(B, C)
    x_cat = x_cat + f32c(inputs["bv"])[None, :]
    xcls = x_cat @ f32c(inputs["Wp"]) + f32c(inputs["bp"])[None, :]
    xcls = xcls.astype(np.float32).reshape(B, 1, C)
    attn = attn.reshape(B, H, 1, N)
    return (xcls, attn), res


def kernel(**inputs):
    outs, _ = _run(inputs)
    return outs


# revision 37
# speedup vs baseline: 1.0459x; 1.0459x over previous
"""ClassAttention (CaiT-style) Trainium2 Bass kernel.

Problem: B=32, N=4096, C=768, H=12, D=64.
  q = (x[:,0] @ Wq + bq) * D**-0.5          (CLS token only)
  k = x @ Wk + bk ; v = x @ Wv + bv
  attn = softmax(q k^T) ; x_cls = (attn v) @ Wp + bp
Returns (x_cls [B,1,C], attn [B,H,1,N]).

Algebraic restructuring (single query per (b,h) means k/v never need to be
materialized):
  scores[b,h,n] = x[b,n] . R[b][:,h]  with  R[b] = Wk @ Qmat[b]   (768 x 12)
      (the bk contribution is constant per (b,h) row -> softmax-invariant)
  x_cls[b,h]    = (sum_n p[b,h,n] x[b,n]) @ Wv[:,hD:(h+1)D] + bv_h
This reduces ~309 GFLOP of dense QKV matmuls to ~5 GFLOP; the kernel is
memory-bound on streaming x once (402 MB).

Sharding: pure data-parallel over batch, 4 batches per core x 8 cores.

Per-core dataflow (2-stage software pipeline over 512-token blocks):
  stage A: DMA x block [128,4,768] -> PE transposes (fp32r identity matmuls,
           full PE rate) -> PSUM -> DVE/ACT copies -> xT [128,6,512]
  stage B: scores s^T = R^T @ xT (fp32r, F=512) -> [12,512] PSUM -> ACT exp
           (no max-sub needed; |s| < ~2 by construction) -> pu [12,4096] +
           per-block partial sums; PE transposes pu tiles -> puT [128,12] ->
           weighted-sum matmuls (puT stationary, x natural moving, fp32r)
           accumulate w [12,768] in PSUM across the batch.
  batch tail: DMA out unnormalized attn + denominators + raw w.
Host epilogue (in kernel()): attn normalization and the tiny per-head
Wv / Wp projections (~0.3% of total FLOPs) in numpy.

fp32r note: the PE's raw-fp32 single-pass mode (4x faster than fp32's
2x-half-speed-pass scheme for the moving operand). All producers feeding
fp32r matmuls are declared float32r so engines round on write (BIR verifier
requirement). Measured end-to-end rel err ~1.5e-4.
"""

import numpy as np

B, N, C = 32, 4096, 768
H, D = 12, 64
SCALE = D ** -0.5
NCORES = 8
NB = B // NCORES          # batches per core = 4
CCH = C // 128            # channel chunks = 6
NT = N // 128             # token tiles per batch = 32
BLK = 512                 # scores block (tokens)
NBLK = N // BLK           # 8
TPB = BLK // 128          # token tiles per block = 4


def _build_nc():
    import concourse.bass as bass
    import concourse.tile as tile
    from concourse import bacc, mybir
    from concourse.masks import make_identity
    from contextlib import ExitStack

    f32 = mybir.dt.float32
    f32r = mybir.dt.float32r
    AF = mybir.ActivationFunctionType

    nc = bacc.Bacc("TRN2", debug=False, target_bir_lowering=False)

    x_d = nc.dram_tensor("x", [NB, N, C], f32r, kind="ExternalInput").ap()
    wq_d = nc.dram_tensor("Wq", [C, C], f32, kind="ExternalInput").ap()
    wk_d = nc.dram_tensor("Wk", [C, C], f32, kind="ExternalInput").ap()
    bq_d = nc.dram_tensor("bq", [C], f32, kind="ExternalInput").ap()
    attn_d = nc.dram_tensor("out_attn", [NB, H, N], f32, kind="ExternalOutput").ap()
    sums_d = nc.dram_tensor("out_sums", [NB, H], f32, kind="ExternalOutput").ap()
    w_d = nc.dram_tensor("out_w", [NB, H, C], f32, kind="ExternalOutput").ap()

    with tile.TileContext(nc) as tc, ExitStack() as ctx:
        const = ctx.enter_context(tc.tile_pool(name="const", bufs=1))

        ident = const.tile([128, 128], f32)
        make_identity(nc, ident)
        identr = const.tile([128, 128], f32r)
        nc.vector.tensor_copy(out=identr, in_=ident)

        bq_sb = const.tile([128, CCH], f32)

        r_sb = const.tile([128, CCH, NB * H], f32r)    # R for all 4 batches (fp32r for fast PE path)

        # ---------------- init: q, Qmat, Wk^T, R ----------------
        with (
            tc.tile_pool(name="init", bufs=1) as initp,
            tc.tile_pool(name="init_ps", bufs=1, space="PSUM") as initps,
            tc.tile_pool(name="init_tps", bufs=2, space="PSUM") as inittps,
        ):
            x0n_sb = initp.tile([NB, C], f32r)
            nc.sync.dma_start(out=x0n_sb, in_=x_d[:, 0, :])
            nc.sync.dma_start(out=bq_sb, in_=bq_d.rearrange("(c p) -> p c", p=128))
            wk_sb = initp.tile([128, CCH, C], f32)
            nc.sync.dma_start(out=wk_sb, in_=wk_d.rearrange("(c p) j -> p c j", p=128))
            wq_sb = initp.tile([128, CCH, C], f32)
            nc.sync.dma_start(out=wq_sb, in_=wq_d.rearrange("(c p) j -> p c j", p=128))
            x0_sb = initp.tile([128, CCH, NB], f32)
            for cc in range(CCH):
                t0 = inittps.tile([128, NB], f32r, tag="tps", name=f"t0_{cc}")
                nc.tensor.transpose(t0, x0n_sb[:, cc * 128:(cc + 1) * 128], identr[:NB, :NB])
                nc.vector.tensor_copy(out=x0_sb[:, cc, :], in_=t0)

            # Wk^T: wkT[p, jc, i] = Wk[i, jc*128+p]
            wkT_sb = initp.tile([128, CCH, C], f32)
            for ic in range(CCH):
                for jc in range(CCH):
                    tp = inittps.tile([128, 128], f32, tag="tps")
                    nc.tensor.transpose(tp, wk_sb[:, ic, jc * 128:(jc + 1) * 128], ident)
                    nc.vector.tensor_copy(out=wkT_sb[:, jc, ic * 128:(ic + 1) * 128], in_=tp)

            # q^T[p, jc, b] = (x0 @ Wq + bq) * SCALE
            q_ps = initps.tile([128, CCH, NB], f32, tag="qps")
            for jc in range(CCH):
                for ic in range(CCH):
                    nc.tensor.matmul(
                        q_ps[:, jc, :],
                        wq_sb[:, ic, jc * 128:(jc + 1) * 128],
                        x0_sb[:, ic, :],
                        start=(ic == 0),
                        stop=(ic == CCH - 1),
                    )
            q_sb = initp.tile([128, CCH, NB], f32)
            for jc in range(CCH):
                nc.vector.tensor_scalar(
                    out=q_sb[:, jc, :],
                    in0=q_ps[:, jc, :],
                    scalar1=bq_sb[:, jc:jc + 1],
                    scalar2=SCALE,
                    op0=mybir.AluOpType.add,
                    op1=mybir.AluOpType.mult,
                )

            # Qmat[p, jc, b*H+h] = q_sb[p, jc, b] if (jc*128+p) in head-h block else 0
            qm_sb = initp.tile([128, CCH, NB * H], f32)
            nc.vector.memset(qm_sb, 0.0)
            for b in range(NB):
                for h in range(H):
                    p0 = (h % 2) * 64
                    nc.vector.tensor_copy(
                        out=qm_sb[p0:p0 + 64, h // 2, b * H + h:b * H + h + 1],
                        in_=q_sb[p0:p0 + 64, h // 2, b:b + 1],
                    )

            # R[p, ic, b*H+h] = sum_j Wk[ic*128+p, j] Qmat[j, b*H+h]
            r_ps = initps.tile([128, CCH, NB * H], f32, tag="rps")
            for ic in range(CCH):
                for jc in range(CCH):
                    nc.tensor.matmul(
                        r_ps[:, ic, :],
                        wkT_sb[:, jc, ic * 128:(ic + 1) * 128],
                        qm_sb[:, jc, :],
                        start=(jc == 0),
                        stop=(jc == CCH - 1),
                    )
            nc.vector.tensor_copy(out=r_sb, in_=r_ps)

        # ---------------- steady: stream x, scores, exp, weighted sums ----------
        xpool = ctx.enter_context(tc.tile_pool(name="x", bufs=5))
        xtpool = ctx.enter_context(tc.tile_pool(name="xT", bufs=3))
        pupool = ctx.enter_context(tc.tile_pool(name="pu", bufs=2))
        smallp = ctx.enter_context(tc.tile_pool(name="small", bufs=4))
        putpool = ctx.enter_context(tc.tile_pool(name="puts", bufs=4))
        ps_xt = ctx.enter_context(tc.tile_pool(name="ps_xt", bufs=4, space="PSUM"))
        ps_s = ctx.enter_context(tc.tile_pool(name="ps_s", bufs=2, space="PSUM"))
        ps_w = ctx.enter_context(tc.tile_pool(name="ps_w", bufs=1, space="PSUM"))

        batch_state = {}

        def stage_a(b, blk):
            """DMA one 512-token block, PE-transpose it, copy xT to SBUF."""
            xq = xpool.tile([128, TPB, C], f32r, tag="xq", name=f"xq_{b}_{blk}")
            nc.sync.dma_start(
                out=xq,
                in_=x_d[b, blk * BLK:(blk + 1) * BLK, :].rearrange(
                    "(t p) c -> p t c", p=128
                ),
            )
            xt_sb = xtpool.tile(
                [128, CCH, BLK], f32r, tag="xt", name=f"xt_{b}_{blk}"
            )
            for tt in range(TPB):
                for hf in range(2):
                    xt_ps = ps_xt.tile(
                        [128, 3, 128], f32r, tag="xtps", name=f"xtps_{b}_{blk}_{tt}_{hf}"
                    )
                    for k in range(3):
                        cc = hf * 3 + k
                        nc.tensor.transpose(
                            xt_ps[:, k, :], xq[:, tt, cc * 128:(cc + 1) * 128], identr
                        )
                    dst = xt_sb[:, hf * 3:hf * 3 + 3, tt * 128:(tt + 1) * 128]
                    if (blk * TPB + tt) % 2 == 1:
                        nc.scalar.copy(out=dst, in_=xt_ps[:])
                    else:
                        nc.vector.tensor_copy(out=dst, in_=xt_ps)
            return xq, xt_sb

        def stage_b(b, blk, xq, xt_sb):
            """Scores, exp, puT transposes, weighted-sum accumulation."""
            st = batch_state[b]
            pu_sb, psums, w_ps = st["pu"], st["psums"], st["w"]
            s_ps = ps_s.tile([12, BLK], f32, tag="sps", name=f"sps_{b}_{blk}")
            for cc in range(CCH):
                nc.tensor.matmul(
                    s_ps,
                    r_sb[:, cc, b * H:(b + 1) * H],
                    xt_sb[:, cc, :],
                    start=(cc == 0),
                    stop=(cc == CCH - 1),
                )
            nc.scalar.activation(
                out=pu_sb[:, blk * BLK:(blk + 1) * BLK],
                in_=s_ps,
                func=AF.Exp,
                accum_out=psums[:, blk:blk + 1],
            )
            put_ps = ps_s.tile(
                [128, TPB, H], f32, tag="sps", name=f"putps_{b}_{blk}"
            )
            for tt in range(TPB):
                t = blk * TPB + tt
                nc.tensor.transpose(
                    put_ps[:, tt, :], pu_sb[:, t * 128:(t + 1) * 128], ident[:12, :12]
                )
            put_sb = putpool.tile(
                [128, TPB, H], f32r, tag="puts", name=f"puts_{b}_{blk}"
            )
            nc.vector.tensor_copy(out=put_sb, in_=put_ps)
            for tt in range(TPB):
                t = blk * TPB + tt
                for hf in range(2):
                    nc.tensor.matmul(
                        w_ps[hf],
                        put_sb[:, tt, :],
                        xq[:, tt, hf * 384:(hf + 1) * 384],
                        start=(t == 0),
                        stop=(t == NT - 1),
                    )

        def batch_tail(b):
            """Unnormalized attn + raw w out (host normalizes and projects),
            plus softmax denominators."""
            st = batch_state.pop(b)
            pu_sb, psums, w_ps = st["pu"], st["psums"], st["w"]
            nc.sync.dma_start(out=attn_d[b], in_=pu_sb)
            s_sum = smallp.tile([12, 1], f32, tag="ssum", name=f"ssum_{b}")
            nc.vector.reduce_sum(out=s_sum, in_=psums, axis=mybir.AxisListType.X)
            nc.sync.dma_start(
                out=sums_d[b:b + 1, :].rearrange("o h -> h o"), in_=s_sum
            )
            w_sb = smallp.tile([12, C], f32, tag="wsb", name=f"wsb_{b}")
            for hf in range(2):
                nc.vector.tensor_copy(
                    out=w_sb[:, hf * 384:(hf + 1) * 384], in_=w_ps[hf]
                )
            nc.sync.dma_start(out=w_d[b], in_=w_sb)

        # 2-stage software pipeline over all (batch, block) pairs: transposes
        # of step i+1 are emitted before scores/wsum of step i, so the PE can
        # run compute for step i while DVE/ACT copies catch up on step i+1.
        steps = [(b, blk) for b in range(NB) for blk in range(NBLK)]
        pend = None
        for b, blk in steps:
            if blk == 0:
                batch_state[b] = {
                    "pu": pupool.tile([12, N], f32, tag="pu", name=f"pu_{b}"),
                    "psums": smallp.tile(
                        [12, NBLK], f32, tag="psums", name=f"psums_{b}"
                    ),
                    "w": [
                        ps_w.tile([12, 384], f32, tag=f"w{i}", name=f"w_ps{i}_{b}")
                        for i in range(2)
                    ],
                }
            cur = (b, blk, *stage_a(b, blk))
            if pend is not None:
                pb, pblk, pxq, pxt = pend
                stage_b(pb, pblk, pxq, pxt)
                if pblk == NBLK - 1:
                    batch_tail(pb)
            pend = cur
        pb, pblk, pxq, pxt = pend
        stage_b(pb, pblk, pxq, pxt)
        batch_tail(pb)

    nc.compile()
    return nc


_NC = None


def _get_nc():
    global _NC
    if _NC is None:
        _NC = _build_nc()
    return _NC


def _run(inputs, trace=False, trace_cores=None):
    from concourse.bass_utils import run_bass_kernel_spmd

    def f32c(a):
        return np.ascontiguousarray(np.asarray(a, dtype=np.float32))

    x = f32c(inputs["x"])
    dev_weights = {k: f32c(inputs[k]) for k in ("Wq", "Wk", "bq")}

    nc = _get_nc()
    in_maps = []
    for i in range(NCORES):
        m = {"x": np.ascontiguousarray(x[i * NB:(i + 1) * NB])}
        m.update(dev_weights)
        in_maps.append(m)
    kwargs = {}
    if trace:
        kwargs.update(trace=True, trace_cores=trace_cores or [0])
    res = run_bass_kernel_spmd(nc, in_maps, core_ids=list(range(NCORES)), **kwargs)
    attn = np.concatenate([r["out_attn"] for r in res.results], axis=0)
    sums = np.concatenate([r["out_sums"] for r in res.results], axis=0)
    w_raw = np.concatenate([r["out_w"] for r in res.results], axis=0)
    attn = attn / sums[:, :, None]
    # host epilogue: per-head Wv projection + final Wp projection (tiny FLOPs)
    w_norm = w_raw / sums[:, :, None]                      # [B, H, C]
    wv = f32c(inputs["Wv"]).reshape(C, H, D)
    x_cat = np.einsum("bhc,chd->bhd", w_norm, wv).reshape(B, C)
    x_cat = x_cat + f32c(inputs["bv"])[None, :]
    xcls = x_cat @ f32c(inputs["Wp"]) + f32c(inputs["bp"])[None, :]
    xcls = xcls.astype(np.float32).reshape(B, 1, C)
    attn = attn.reshape(B, H, 1, N)
    return (xcls, attn), res


def kernel(**inputs):
    outs, _ = _run(inputs)
    return outs
